# revision 44
# baseline (speedup 1.0000x reference)
"""Trainium2 Bass kernel for nn_Attention_4930622456197.

Multi-head causal attention (B=2, S=2048, D=2048, 32 heads x head_dim 64)
with QKVO projections, tensor-parallel over heads across 8 NeuronCores
(4 heads per core).

Per-core plan (all matmul inputs bf16, f32 PSUM accumulation):
  Phase 1  QKV projections from host-pretransposed x^T [D, T]:
           Q^T, K^T in [128 (=2 heads x 64 dims), group, T] layout;
           V in natural [tok, head, 65] layout with a ones column
           appended (row 64 of V_aug.T) so the P@V matmul also produces
           the softmax denominators for free.
  Phase 2  Flash-style causal attention in score-transposed layout
           S^T[s, q] (scores never touch HBM).  exp on ScalarE with the
           1/sqrt(hd) scale folded in; no max-subtraction (scores are
           O(+-8) here, exp is safe in fp32->bf16).  The diagonal
           128x128 block of each strip is masked post-exp with a
           precomputed upper-triangular 0/1 tile.  O^T accumulates in
           PSUM over k-tiles; the 64 ones-columns of V_aug broadcast the
           softmax denominator to PSUM partitions 64..127, and 1/den is
           computed with a DVE reciprocal (keeping ScalarE a pure exp
           stream).
  Phase 3  Row-parallel output projection producing a partial
           out^T [D, T]; host sums the 8 partials, adds wo_b.

  Scheduling: attention runs as 16 regions (batch x q-chunk x head-pair
  stream).  QKV chunks and output-projection tiles are "filler" thunks
  pumped between attention j-steps from a DEDICATED 2-bank PSUM pool so
  they never serialize against the score->exp->PV chain (which owns its
  own 4 banks + 2 accumulator banks).  Startup DMAs are issued from
  four different engine queues so descriptor generation parallelizes.

The harness calls kernel(**inputs) with the full (unsharded) inputs and
expects the full [2, 2048, 2048] float32 output.
"""

import numpy as np
import ml_dtypes

BSZ, SEQ, DIM, NH = 2, 2048, 2048, 32
HD = DIM // NH            # 64
NCORES = 8
HPC = NH // NCORES        # 4 heads per core
HSL = HPC * HD            # 256 head-dims per core
T = BSZ * SEQ             # 4096 flattened tokens
SCALE = 1.0 / float(np.sqrt(HD))
BF16 = ml_dtypes.bfloat16

NKT = DIM // 128          # 16 contraction tiles over model dim
NCH = T // 512            # 8 token chunks of 512
NJ = SEQ // 128           # 16 k-tiles per sequence
NCK = SEQ // 512          # 4 q-chunks per sequence

# Output partial dtype: float32 is safest for the cross-core sum;
# bfloat16 halves the output DMA traffic.
OUT_BF16 = True

LAST_RESULTS = None       # BassKernelResults of the most recent run (for test.py)


# This walrus build caps EVERY instruction (HW-decoded and sequencer alike)
# at one sync-wait, so the legalizer splits excess waits regardless of opcode.
_SEQ_OPCODES = set()
_wc_counter = [0]


def _legalize_bir_waits(bir_bytes):
    """This container's walrus accepts only ONE sync-wait on HW-decoded
    instruction structs ("Too many sync wait commands" otherwise), but Tile
    freely emits 2-3 waits per instruction.  Split excess waits into
    standalone same-engine EventSemaphore instructions placed immediately
    before the instruction — the sequencer executes them in order, so the
    dependency semantics are identical."""
    import json as _json

    d = _json.loads(bir_bytes)
    n_split = 0
    for f in d.get("functions", []):
        for blk in f.get("blocks", []):
            out = []
            for ins in blk.get("instructions", []):
                si = ins.get("sync_info")
                waits = (si or {}).get("on_wait") or []
                if si is not None and len(waits) > 1 and \
                        ins.get("opcode") not in _SEQ_OPCODES:
                    for w in waits[:-1]:
                        _wc_counter[0] += 1
                        out.append({
                            "debug": ins.get("debug", 0),
                            "engine": ins["engine"],
                            "ins": [], "outs": [],
                            "name": f"I-wc{_wc_counter[0]}",
                            "opcode": "EventSemaphore",
                            "sync_info": {"on_wait": [w], "on_update": []},
                        })
                        n_split += 1
                    si["on_wait"] = waits[-1:]
                out.append(ins)
            blk["instructions"] = out
    if n_split:
        print(f"[kernel] wait-legalizer: split {n_split} excess waits")
    return _json.dumps(d).encode()


_hook_installed = [False]


def _install_compile_hook():
    """Route every BIR->NEFF compile in this process through the wait
    legalizer (both the direct bass_utils path and the bass2jax/axon path)."""
    if _hook_installed[0]:
        return
    import concourse.bass_utils as bu

    orig = bu.compile_bir_kernel

    def patched(bir_json, tmpdir, neff_name="file.neff"):
        return orig(_legalize_bir_waits(bir_json), tmpdir, neff_name=neff_name)

    bu.compile_bir_kernel = patched
    try:
        import concourse.bass2jax as b2j
        b2j.compile_bir_kernel = patched
    except Exception:
        pass
    _hook_installed[0] = True


def _build(mask_mode, use_qkb, use_vb):
    """Builds the Bass program. mask_mode: 'causal' | 'none' | 'general'."""
    import functools
    import concourse.bass as bass
    import concourse.mybir as mybir
    import concourse.tile as tile
    from concourse.masks import make_upper_triangular

    dt = mybir.dt
    f32 = dt.float32
    bf16 = dt.bfloat16
    Exp = mybir.ActivationFunctionType.Exp
    Ln = mybir.ActivationFunctionType.Ln
    Identity = mybir.ActivationFunctionType.Identity
    out_dt = bf16 if OUT_BF16 else f32

    causal = mask_mode == "causal"

    nc = bass.Bass()
    xT_d = nc.dram_tensor("xt", [DIM, T], bf16, kind="ExternalInput")
    wqT_d = nc.dram_tensor("wqt", [DIM, HSL], bf16, kind="ExternalInput")
    wkT_d = nc.dram_tensor("wkt", [DIM, HSL], bf16, kind="ExternalInput")
    wvT_d = nc.dram_tensor("wvt", [DIM, HSL], bf16, kind="ExternalInput")
    woT_d = nc.dram_tensor("wot", [HSL, DIM], bf16, kind="ExternalInput")
    outT_d = nc.dram_tensor("outT", [DIM, T], out_dt, kind="ExternalOutput")
    qb_d = kb_d = vb_d = maskT_d = None
    if use_qkb:
        qb_d = nc.dram_tensor("qb", [HSL], f32, kind="ExternalInput")
        kb_d = nc.dram_tensor("kb", [HSL], f32, kind="ExternalInput")
    if use_vb:
        vb_d = nc.dram_tensor("vb", [HSL], f32, kind="ExternalInput")
    if mask_mode == "general":
        maskT_d = nc.dram_tensor("maskt", [SEQ, SEQ], f32, kind="ExternalInput")

    # 3-D views with 128-partition-major layout
    xT_ap = xT_d[:].rearrange("(kt p) t -> p kt t", p=128)
    wq_ap = wqT_d[:].rearrange("(kt p) m -> p kt m", p=128)
    wk_ap = wkT_d[:].rearrange("(kt p) m -> p kt m", p=128)
    wv_ap = wvT_d[:].rearrange("(kt p) m -> p kt m", p=128)
    wo_ap = woT_d[:].rearrange("(g p) n -> p g n", p=128)
    outT_ap = outT_d[:].rearrange("(ot p) t -> p ot t", p=128)

    with tile.TileContext(nc) as tc:
        with (
            tc.tile_pool(name="singles", bufs=1) as singles,
            tc.tile_pool(name="xload", bufs=3) as xload,
            tc.tile_pool(name="work", bufs=4) as work,
            tc.tile_pool(name="outp", bufs=4) as outp,
            tc.tile_pool(name="attps", bufs=2, space="PSUM") as attps,
            tc.tile_pool(name="otps", bufs=2, space="PSUM") as otps,
            tc.tile_pool(name="filps", bufs=2, space="PSUM") as filps,
        ):
            # ---- resident tensors -------------------------------------
            wq_sb = singles.tile([128, NKT, HSL], bf16)
            wk_sb = singles.tile([128, NKT, HSL], bf16)
            wv_sb = singles.tile([128, NKT, HSL], bf16)
            wo_sb = singles.tile([128, 2, DIM], bf16)

            qt_sb = singles.tile([128, 2, T], bf16)
            kt_sb = singles.tile([128, 2, T], bf16)
            ctxT_sb = singles.tile([128, 2, T], bf16)
            # V with 64 ones-columns per head: the P@V matmul then writes the
            # softmax denominator to PSUM partitions 64..127 (a free
            # cross-partition broadcast).
            vaug_sb = singles.tile([128, T // 128, HPC, 2 * HD], bf16)

            # Startup-critical DMAs.  The first compute unit (Q-proj, group
            # 0) needs (wq quarter i, x0 quarter i) pairs in order, so the
            # descriptor generation for the first two pairs is spread over
            # four otherwise-idle engine queues; the rest go on SP.
            xt0 = xload.tile([128, NKT, 512], bf16, tag="xt")
            # first k-pair split in half so the very first matmul's inputs
            # (wq k0-1, x0 k0-1) arrive ahead of the bulk transfers
            for ksl in (slice(0, 2), slice(2, 4)):
                nc.sync.dma_start(out=wq_sb[:, ksl], in_=wq_ap[:, ksl])
                nc.sync.dma_start(out=xt0[:, ksl], in_=xT_ap[:, ksl, 0:512])
            for q in range(1, 4):
                ksl = slice(4 * q, 4 * q + 4)
                nc.sync.dma_start(out=wq_sb[:, ksl], in_=wq_ap[:, ksl])
                nc.sync.dma_start(out=xt0[:, ksl], in_=xT_ap[:, ksl, 0:512])
            for q in range(4):
                ksl = slice(4 * q, 4 * q + 4)
                nc.sync.dma_start(out=wk_sb[:, ksl], in_=wk_ap[:, ksl])
            for q in range(4):
                ksl = slice(4 * q, 4 * q + 4)
                nc.sync.dma_start(out=wv_sb[:, ksl], in_=wv_ap[:, ksl])
            nc.sync.dma_start(out=wo_sb, in_=wo_ap)



            qb_sb = kb_sb = vb_bc = None
            if use_qkb:
                qb_sb = singles.tile([128, 2], f32)
                kb_sb = singles.tile([128, 2], f32)
                nc.sync.dma_start(out=qb_sb, in_=qb_d[:].rearrange("(g p) -> p g", p=128))
                nc.sync.dma_start(out=kb_sb, in_=kb_d[:].rearrange("(g p) -> p g", p=128))
            if use_vb:
                vb_bc = singles.tile([128, HSL], f32)
                nc.sync.dma_start(out=vb_bc, in_=vb_d[:].to_broadcast([128, HSL]))

            triu_sb = None
            if causal:
                triu_sb = singles.tile([128, 128], bf16)
                make_upper_triangular(nc, triu_sb, val=1.0, diag=True)
                # ~3.4us of dummy matmuls during the initial DMA wait: trips
                # the HAM activity window so the PE is already at 2.4 GHz
                # when the first projection matmuls arrive.
                warm_ps = filps.tile([128, 512], f32, tag="fil", name="warm")
                for w in range(56):
                    nc.tensor.matmul(
                        warm_ps[:, 0:128], lhsT=triu_sb[0:64, :],
                        rhs=triu_sb[0:64, :],
                        start=(w == 0), stop=(w == 55), tile_position=(0, 0))

            # ones columns of V_aug, written once on the DVE (idle at start;
            # keeping it OFF the Pool queue lets the triangular mask finish
            # early so the PE warm-up matmuls fire during the DMA wait)
            nc.vector.memset(vaug_sb[:, :, :, HD:2 * HD], 1.0)

            # ---- filler units -----------------------------------------
            # QKV projections and the output projection are emitted as
            # "filler" thunks interleaved between attention j-steps.  They
            # draw PSUM from their OWN 2-bank pool so they never serialize
            # against the score->exp->PV chain.

            xt_tiles = {0: xt0}

            def load_unit(ch):
                tsl = slice(ch * 512, (ch + 1) * 512)
                xt_ch = xload.tile([128, NKT, 512], bf16, tag="xt")
                for q in range(4):
                    ksl = slice(4 * q, 4 * q + 4)
                    nc.sync.dma_start(out=xt_ch[:, ksl], in_=xT_ap[:, ksl, tsl])
                xt_tiles[ch] = xt_ch

            def qk_unit(ch, w_sb, dst_sb, b_sb, g):
                tsl = slice(ch * 512, (ch + 1) * 512)
                ps = filps.tile([128, 512], f32, tag="fil", name="qk")
                for k in range(NKT):
                    nc.tensor.matmul(
                        ps, lhsT=w_sb[:, k, g * 128:(g + 1) * 128],
                        rhs=xt_tiles[ch][:, k, :],
                        start=(k == 0), stop=(k == NKT - 1))
                if b_sb is not None:
                    nc.scalar.activation(
                        out=dst_sb[:, g, tsl], in_=ps,
                        func=Identity, bias=b_sb[:, g:g + 1], scale=1.0)
                else:
                    nc.vector.tensor_copy(out=dst_sb[:, g, tsl], in_=ps)

            def v_unit(ch, tp):
                ps = filps.tile([128, 512], f32, tag="fil", name="v")
                for i in range(2):
                    tt = 2 * tp + i
                    for k in range(NKT):
                        nc.tensor.matmul(
                            ps[:, i * HSL:(i + 1) * HSL],
                            lhsT=xt_tiles[ch][:, k, tt * 128:(tt + 1) * 128],
                            rhs=wv_sb[:, k, :],
                            start=(k == 0), stop=(k == NKT - 1))
                tg0 = ch * 4 + 2 * tp
                vdst = vaug_sb[:, tg0:tg0 + 2, :, 0:HD]
                vsrc = ps.rearrange("p (i h m) -> p i h m", i=2, h=HPC)
                if vb_bc is not None:
                    nc.vector.tensor_add(
                        out=vdst, in0=vsrc,
                        in1=vb_bc[:, None, :].to_broadcast(
                            [128, 2, HSL]).rearrange(
                            "p i (h m) -> p i h m", h=HPC))
                else:
                    nc.vector.tensor_copy(out=vdst, in_=vsrc)

            def qkv_units(ch, with_load=True):
                """Thunk list for one 512-token chunk of QKV projection."""
                th = []
                if with_load:
                    th.append(functools.partial(load_unit, ch))
                for g in range(2):
                    th.append(functools.partial(qk_unit, ch, wq_sb, qt_sb, qb_sb, g))
                for g in range(2):
                    th.append(functools.partial(qk_unit, ch, wk_sb, kt_sb, kb_sb, g))
                for tp in range(2):
                    th.append(functools.partial(v_unit, ch, tp))
                return th

            osb_box = {}

            def o_unit(ch, o, vec_evict=False, dma_eng=None):
                tsl = slice(ch * 512, (ch + 1) * 512)
                ps = filps.tile([128, 512], f32, tag="fil", name="o")
                for g2 in range(2):
                    nc.tensor.matmul(
                        ps, lhsT=wo_sb[:, g2, o * 128:(o + 1) * 128],
                        rhs=ctxT_sb[:, g2, tsl],
                        start=(g2 == 0), stop=(g2 == 1))
                if o % 2 == 0:
                    osb_box[ch] = outp.tile([128, 2, 512], out_dt,
                                            tag="out_sb", name="osb")
                osb = osb_box[ch]
                if vec_evict or o % 4 < 2:
                    nc.vector.tensor_copy(out=osb[:, o % 2], in_=ps)
                else:
                    nc.scalar.copy(out=osb[:, o % 2], in_=ps)
                if o % 2 == 1:
                    (dma_eng or nc.sync).dma_start(
                        out=outT_ap[:, o - 1:o + 1, tsl], in_=osb)

            def o_pair2(ch, op, vec_evict=False, dma_eng=None):
                """Two output row-blocks on one [128,1024] PSUM tile from the
                attention pool — used only in the final drain, when the
                score pool is free, to widen the eviction rotation."""
                tsl = slice(ch * 512, (ch + 1) * 512)
                ps2 = attps.tile([128, 1024], f32, tag="st2", name="o2")
                for i in range(2):
                    o = 2 * op + i
                    for g2 in range(2):
                        nc.tensor.matmul(
                            ps2[:, i * 512:(i + 1) * 512],
                            lhsT=wo_sb[:, g2, o * 128:(o + 1) * 128],
                            rhs=ctxT_sb[:, g2, tsl],
                            start=(g2 == 0), stop=(g2 == 1))
                osb = outp.tile([128, 2, 512], out_dt, tag="out_sb", name="osb")
                src2 = ps2.rearrange("p (i n) -> p i n", i=2)
                if vec_evict:
                    nc.vector.tensor_copy(out=osb, in_=src2)
                else:
                    nc.scalar.copy(out=osb, in_=src2)
                (dma_eng or nc.sync).dma_start(
                    out=outT_ap[:, 2 * op:2 * op + 2, tsl], in_=osb)

            def oproj_units(ch, vec_evict=False):
                return [functools.partial(o_unit, ch, o, vec_evict)
                        for o in range(DIM // 128)]

            def oproj_drain(ch):
                """Final chunk: the g2=0 half of the first six row-blocks
                only needs head-group 0's context (ready one region early),
                so those matmuls run while the last region's reciprocal
                chain produces head-group 1's context.  PSUM draws from
                both the filler pool and the (now idle) attention pool."""
                tsl = slice(ch * 512, (ch + 1) * 512)
                a = filps.tile([128, 512], f32, tag="fil", name="da")
                b = filps.tile([128, 512], f32, tag="fil", name="db")
                c2 = attps.tile([128, 1024], f32, tag="st2", name="dc")
                d2 = attps.tile([128, 1024], f32, tag="st2", name="dd")
                slots = [(a, [0]), (b, [1]), (c2, [2, 3]), (d2, [4, 5])]
                for g2 in range(2):
                    for ps, olist in slots:
                        for idx, o in enumerate(olist):
                            dst = ps[:, idx * 512:(idx + 1) * 512] \
                                if len(olist) > 1 else ps
                            nc.tensor.matmul(
                                dst,
                                lhsT=wo_sb[:, g2, o * 128:(o + 1) * 128],
                                rhs=ctxT_sb[:, g2, tsl],
                                start=(g2 == 0), stop=(g2 == 1))
                osb_ab = outp.tile([128, 2, 512], out_dt, tag="out_sb",
                                   name="osb")
                nc.vector.tensor_copy(out=osb_ab[:, 0], in_=a)
                nc.scalar.copy(out=osb_ab[:, 1], in_=b)
                nc.gpsimd.dma_start(out=outT_ap[:, 0:2, tsl], in_=osb_ab)
                for si, ps in ((0, c2), (1, d2)):
                    osb = outp.tile([128, 2, 512], out_dt, tag="out_sb",
                                    name="osb")
                    src = ps.rearrange("p (i n) -> p i n", i=2)
                    if si == 0:
                        nc.vector.tensor_copy(out=osb, in_=src)
                    else:
                        nc.scalar.copy(out=osb, in_=src)
                    nc.gpsimd.dma_start(
                        out=outT_ap[:, 2 + 2 * si:4 + 2 * si, tsl], in_=osb)
                # remaining row-blocks, ordinary rotation; descriptor
                # generation for the final output DMAs goes to the idle
                # Pool queue so it never serializes behind SP work
                o_unit(ch, 6, True, dma_eng=nc.gpsimd)
                o_unit(ch, 7, False, dma_eng=nc.gpsimd)
                o_pair2(ch, 4, True, dma_eng=nc.gpsimd)
                o_pair2(ch, 5, False, dma_eng=nc.gpsimd)
                o_pair2(ch, 6, True, dma_eng=nc.gpsimd)
                o_pair2(ch, 7, False, dma_eng=nc.gpsimd)

            def pump(filler, n=1):
                for _ in range(n):
                    t = next(filler, None)
                    if t is None:
                        return False
                    t()
                return True

            def drain(filler):
                while pump(filler):
                    pass

            # ---- attention regions ------------------------------------

            pending_ep = [None]

            def att_region(b, c, gg, filler, reserve_n=2, last=False):
                """Attention for one (batch, q-chunk, head-pair): the two
                heads of group gg are row-packed in the score matmuls and
                PSUM tile; O^T flush matmuls lag 3 j-steps; filler thunks
                are spread over the j-steps with 2 reserved to bridge the
                region boundary.  The previous region's 1/den + ctx-scale
                chain is emitted after this region's first exp so it never
                delays the exp stream at the boundary."""
                thunks = list(filler)
                reserve = thunks[-reserve_n:] if len(thunks) > reserve_n else []
                body = thunks[:len(thunks) - len(reserve)]
                bi = [0]
                ots = [otps.tile([128, 512], f32, tag="ot", name="ot")
                       for _ in range(2)]
                jmax = 4 * c + 4 if causal else NJ
                pend = []

                def flush_ot(j, qo, pt2):
                    for hh in range(2):
                        nc.tensor.matmul(
                            ots[hh][:, qo:512],
                            lhsT=vaug_sb[:, b * NJ + j, 2 * gg + hh, :],
                            rhs=pt2[:, 512 * hh + qo:512 * hh + 512],
                            start=(j == 0), stop=(j == jmax - 1))

                for j in range(jmax):
                    qo = max(0, j * 128 - c * 512) if causal else 0
                    ssl = slice(b * SEQ + j * 128, b * SEQ + (j + 1) * 128)
                    qsl = slice(b * SEQ + c * 512 + qo, b * SEQ + (c + 1) * 512)
                    st2 = attps.tile([128, 1024], f32, tag="st2", name="st2")
                    nc.tensor.matmul(
                        st2[:, qo:512], lhsT=kt_sb[0:64, gg, ssl],
                        rhs=qt_sb[0:64, gg, qsl],
                        start=True, stop=True, tile_position=(0, 0))
                    nc.tensor.matmul(
                        st2[:, 512 + qo:1024], lhsT=kt_sb[64:128, gg, ssl],
                        rhs=qt_sb[64:128, gg, qsl],
                        start=True, stop=True, tile_position=(64, 0))
                    if maskT_d is not None:
                        mt = work.tile([128, 512], f32, tag="mt")
                        nc.sync.dma_start(
                            out=mt,
                            in_=maskT_d[j * 128:(j + 1) * 128,
                                        c * 512:(c + 1) * 512])
                        for hh in range(2):
                            sl = slice(512 * hh, 512 * hh + 512)
                            nc.vector.tensor_add(
                                out=st2[:, sl], in0=st2[:, sl], in1=mt)
                    pt2 = work.tile([128, 1024], bf16, tag="pt", bufs=8)
                    nc.scalar.activation(
                        out=pt2.rearrange("p (two n) -> p two n", two=2)[:, :, qo:512],
                        in_=st2.rearrange("p (two n) -> p two n", two=2)[:, :, qo:512],
                        func=Exp, scale=SCALE)
                    if causal and j * 128 >= c * 512:
                        dv = pt2.rearrange("p (two n) -> p two n", two=2)[:, :, qo:qo + 128]
                        nc.vector.tensor_mul(
                            out=dv, in0=dv,
                            in1=triu_sb[:, None, :].to_broadcast([128, 2, 128]))
                    pend.append((j, qo, pt2))
                    if j == 0 and pending_ep[0] is not None:
                        pending_ep[0]()
                        pending_ep[0] = None
                    while len(pend) > (1 if last else 3):
                        flush_ot(*pend.pop(0))
                    # front-load two units so the boundary (where the exp
                    # chain restarts and the deferred recip runs) has PE work
                    want = ((j + 1) * len(body) + jmax - 1) // jmax + 2
                    while bi[0] < min(want, len(body)):
                        body[bi[0]]()
                        bi[0] += 1
                while pend:
                    flush_ot(*pend.pop(0))
                for t in reserve:
                    t()
                # region end: one f32 copy per accumulator frees its PSUM
                # bank immediately; the 1/den = exp(-ln(den)) + ctx scale
                # run later (deferred past the next region's first exp) from
                # SBUF, batched over both heads.
                csl = slice(b * SEQ + c * 512, b * SEQ + (c + 1) * 512)
                un2 = work.tile([128, 2, 512], f32, tag="unctx", bufs=2)
                nc.vector.tensor_copy(out=un2[:, 0], in_=ots[0])
                if last:
                    # parallel eviction across engines shortens the final
                    # serial chain into the output-projection drain
                    nc.scalar.copy(out=un2[:, 1], in_=ots[1])
                else:
                    nc.vector.tensor_copy(out=un2[:, 1], in_=ots[1])

                def epilogue():
                    rb2 = work.tile([64, 2, 512], f32, tag="rb", bufs=2)
                    nc.scalar.activation(out=rb2, in_=un2[HD:2 * HD],
                                         func=Ln, scale=1.0)
                    nc.scalar.activation(out=rb2, in_=rb2, func=Exp, scale=-1.0)
                    for hh in range(2):
                        nc.vector.tensor_mul(
                            out=ctxT_sb[hh * 64:(hh + 1) * 64, gg, csl],
                            in0=un2[0:HD, hh], in1=rb2[:, hh])

                pending_ep[0] = epilogue

            # ---- schedule ---------------------------------------------
            # qkv chunks feed forward (region (b,c) needs chunks <= 4b+c);
            # fine-grained oproj units land in the ScalarE-heavy late
            # regions.  Each entry: (b, c, gg, filler thunks).
            def mix(units, ounits):
                """Interleave o-units between the bigger qkv units so their
                eviction latency hides under the 16-matmul streams."""
                out = []
                per = (len(ounits) + len(units) - 1) // max(len(units), 1)
                oi = 0
                for u in units:
                    out.append(u)
                    for _ in range(per):
                        if oi < len(ounits):
                            out.append(ounits[oi])
                            oi += 1
                out.extend(ounits[oi:])
                return out

            q1, q2, q3 = qkv_units(1), qkv_units(2), qkv_units(3)
            q4, q5 = qkv_units(4), qkv_units(5)
            q6, q7 = qkv_units(6), qkv_units(7)
            o0, o1 = oproj_units(0), oproj_units(1)
            o2 = oproj_units(2, vec_evict=True)
            o3 = oproj_units(3, vec_evict=True)
            o4 = oproj_units(4, vec_evict=True)
            o5 = oproj_units(5, vec_evict=True)
            o6 = oproj_units(6, vec_evict=True)
            # chunk X's ctx epilogue is deferred into the region AFTER X's
            # last one, so oproj(X) units are placed two or more regions
            # after region X to avoid stalling the in-order PE queue.
            regions = [
                (0, 0, 0, q1[:4], 2),
                (0, 0, 1, q1[4:], 2),
                (0, 1, 0, q2[:4], 2),
                (0, 1, 1, mix(q2[4:], o0[:8]), 2),
                (0, 2, 0, mix(q3[:4], o0[8:]), 2),
                (0, 2, 1, mix(q3[4:] + q4[:1], o1[:8]), 2),
                (0, 3, 0, mix(q4[1:4], o1[8:]), 2),
                (0, 3, 1, mix(q4[4:] + q5[:2], o2[:8]), 2),
                (1, 0, 0, mix(q5[2:5], o2[8:]), 2),
                (1, 0, 1, q5[5:] + q6[:2], 2),
                (1, 1, 0, q6[2:], 2),
                (1, 1, 1, q7, 2),
                (1, 2, 0, o3 + o4[:8], 4),
                (1, 2, 1, o4[8:] + o5[:8], 4),
                (1, 3, 0, o5[8:] + o6[:4], 4),
                (1, 3, 1, o6[4:10], 2),
            ]
            drain(iter(qkv_units(0, with_load=False)))
            for ri, (b, c, gg, filler, rn) in enumerate(regions):
                att_region(b, c, gg, filler, reserve_n=rn,
                           last=(ri == len(regions) - 1))
            # the last region's recip chain (emitted first so its DVE muls
            # sit ahead of the bridge evictions in the Vector FIFO) overlaps
            # the leftover chunk-6 units and the g2=0 half of the chunk-7
            # drain, neither of which depends on it.
            if pending_ep[0] is not None:
                pending_ep[0]()
                pending_ep[0] = None
            if causal:
                # Dummy matmuls into an accumulator-pool tile: the pool
                # rotation makes them wait for the epilogue's eviction copy,
                # so they execute exactly inside the recip-chain bubble and
                # keep the PE's HAM activity window warm through it.
                dummy = otps.tile([128, 512], f32, tag="ot", name="hamwarm")
                for w in range(64):
                    nc.tensor.matmul(
                        dummy[:, 0:128], lhsT=triu_sb[0:64, :],
                        rhs=triu_sb[0:64, :],
                        start=(w == 0), stop=(w == 63), tile_position=(0, 0))
            for t in o6[10:]:
                t()
            oproj_drain(7)

    return nc


def _classify_mask(mask):
    m = np.asarray(mask, dtype=np.float32).reshape(SEQ, SEQ)
    if not np.any(m):
        return "none", None
    lower_ok = not np.any(m[np.tril_indices(SEQ)])
    upper = m[np.triu_indices(SEQ, 1)]
    if lower_ok and np.all(np.isneginf(upper)):
        return "causal", None
    return "general", np.ascontiguousarray(m.T)


def kernel(x, start_pos, freqs_cis, mask, wq_w, wq_b, wk_w, wk_b,
           wv_w, wv_b, wo_w, wo_b):
    global LAST_RESULTS
    _install_compile_hook()
    from concourse.bass_utils import run_bass_kernel_spmd

    x = np.asarray(x, dtype=np.float32)
    mask_mode, maskT = _classify_mask(mask)
    wq_b = np.asarray(wq_b, dtype=np.float32)
    wk_b = np.asarray(wk_b, dtype=np.float32)
    wv_b = np.asarray(wv_b, dtype=np.float32)
    wo_b = np.asarray(wo_b, dtype=np.float32)
    use_qkb = bool(np.any(wq_b) or np.any(wk_b))
    use_vb = bool(np.any(wv_b))

    nc = _build(mask_mode, use_qkb, use_vb)

    xT = np.ascontiguousarray(x.reshape(T, DIM).T).astype(BF16)
    wqT = np.asarray(wq_w, dtype=np.float32).T.astype(BF16)  # [D, D]
    wkT = np.asarray(wk_w, dtype=np.float32).T.astype(BF16)
    wvT = np.asarray(wv_w, dtype=np.float32).T.astype(BF16)
    wo = np.asarray(wo_w, dtype=np.float32)

    in_maps = []
    for c in range(NCORES):
        sl = slice(HSL * c, HSL * (c + 1))
        im = {
            "xt": xT,
            "wqt": np.ascontiguousarray(wqT[:, sl]),
            "wkt": np.ascontiguousarray(wkT[:, sl]),
            "wvt": np.ascontiguousarray(wvT[:, sl]),
            "wot": np.ascontiguousarray(wo[:, sl].T).astype(BF16),
        }
        if use_qkb:
            im["qb"] = np.ascontiguousarray(wq_b[sl])
            im["kb"] = np.ascontiguousarray(wk_b[sl])
        if use_vb:
            im["vb"] = np.ascontiguousarray(wv_b[sl])
        if mask_mode == "general":
            im["maskt"] = maskT
        in_maps.append(im)

    res = run_bass_kernel_spmd(nc, in_maps, core_ids=list(range(NCORES)))
    LAST_RESULTS = res

    acc = np.zeros((DIM, T), dtype=np.float32)
    for r in res.results:
        acc += np.asarray(r["outT"], dtype=np.float32)
    out = acc.T + wo_b[None, :]
    return out.reshape(BSZ, SEQ, DIM).astype(np.float32)


# revision 48
# speedup vs baseline: 1.0062x; 1.0062x over previous
"""Trainium2 Bass kernel for nn_Attention_4930622456197.

Multi-head causal attention (B=2, S=2048, D=2048, 32 heads x head_dim 64)
with QKVO projections, tensor-parallel over heads across 8 NeuronCores
(4 heads per core).

Per-core plan (all matmul inputs bf16, f32 PSUM accumulation):
  Phase 1  QKV projections from host-pretransposed x^T [D, T]:
           Q^T, K^T in [128 (=2 heads x 64 dims), group, T] layout;
           V in natural [tok, head, 65] layout with a ones column
           appended (row 64 of V_aug.T) so the P@V matmul also produces
           the softmax denominators for free.
  Phase 2  Flash-style causal attention in score-transposed layout
           S^T[s, q] (scores never touch HBM).  exp on ScalarE with the
           1/sqrt(hd) scale folded in; no max-subtraction (scores are
           O(+-8) here, exp is safe in fp32->bf16).  The diagonal
           128x128 block of each strip is masked post-exp with a
           precomputed upper-triangular 0/1 tile.  O^T accumulates in
           PSUM over k-tiles; the 64 ones-columns of V_aug broadcast the
           softmax denominator to PSUM partitions 64..127, and 1/den is
           computed with a DVE reciprocal (keeping ScalarE a pure exp
           stream).
  Phase 3  Row-parallel output projection producing a partial
           out^T [D, T]; host sums the 8 partials, adds wo_b.

  Scheduling: attention runs as 16 regions (batch x q-chunk x head-pair
  stream).  QKV chunks and output-projection tiles are "filler" thunks
  pumped between attention j-steps from a DEDICATED 2-bank PSUM pool so
  they never serialize against the score->exp->PV chain (which owns its
  own 4 banks + 2 accumulator banks).  Startup DMAs are issued from
  four different engine queues so descriptor generation parallelizes.

The harness calls kernel(**inputs) with the full (unsharded) inputs and
expects the full [2, 2048, 2048] float32 output.
"""

import numpy as np
import ml_dtypes

BSZ, SEQ, DIM, NH = 2, 2048, 2048, 32
HD = DIM // NH            # 64
NCORES = 8
HPC = NH // NCORES        # 4 heads per core
HSL = HPC * HD            # 256 head-dims per core
T = BSZ * SEQ             # 4096 flattened tokens
SCALE = 1.0 / float(np.sqrt(HD))
BF16 = ml_dtypes.bfloat16

NKT = DIM // 128          # 16 contraction tiles over model dim
NCH = T // 512            # 8 token chunks of 512
NJ = SEQ // 128           # 16 k-tiles per sequence
NCK = SEQ // 512          # 4 q-chunks per sequence

# Output partial dtype: float32 is safest for the cross-core sum;
# bfloat16 halves the output DMA traffic.
OUT_BF16 = True

LAST_RESULTS = None       # BassKernelResults of the most recent run (for test.py)


# This walrus build caps EVERY instruction (HW-decoded and sequencer alike)
# at one sync-wait, so the legalizer splits excess waits regardless of opcode.
_SEQ_OPCODES = set()
_wc_counter = [0]


def _legalize_bir_waits(bir_bytes):
    """This container's walrus accepts only ONE sync-wait on HW-decoded
    instruction structs ("Too many sync wait commands" otherwise), but Tile
    freely emits 2-3 waits per instruction.  Split excess waits into
    standalone same-engine EventSemaphore instructions placed immediately
    before the instruction — the sequencer executes them in order, so the
    dependency semantics are identical."""
    import json as _json

    d = _json.loads(bir_bytes)
    n_split = 0
    for f in d.get("functions", []):
        for blk in f.get("blocks", []):
            out = []
            for ins in blk.get("instructions", []):
                si = ins.get("sync_info")
                waits = (si or {}).get("on_wait") or []
                if si is not None and len(waits) > 1 and \
                        ins.get("opcode") not in _SEQ_OPCODES:
                    for w in waits[:-1]:
                        _wc_counter[0] += 1
                        out.append({
                            "debug": ins.get("debug", 0),
                            "engine": ins["engine"],
                            "ins": [], "outs": [],
                            "name": f"I-wc{_wc_counter[0]}",
                            "opcode": "EventSemaphore",
                            "sync_info": {"on_wait": [w], "on_update": []},
                        })
                        n_split += 1
                    si["on_wait"] = waits[-1:]
                out.append(ins)
            blk["instructions"] = out
    if n_split:
        print(f"[kernel] wait-legalizer: split {n_split} excess waits")
    return _json.dumps(d).encode()


_hook_installed = [False]


def _install_compile_hook():
    """Route every BIR->NEFF compile in this process through the wait
    legalizer (both the direct bass_utils path and the bass2jax/axon path)."""
    if _hook_installed[0]:
        return
    import concourse.bass_utils as bu

    orig = bu.compile_bir_kernel

    def patched(bir_json, tmpdir, neff_name="file.neff"):
        return orig(_legalize_bir_waits(bir_json), tmpdir, neff_name=neff_name)

    bu.compile_bir_kernel = patched
    try:
        import concourse.bass2jax as b2j
        b2j.compile_bir_kernel = patched
    except Exception:
        pass
    _hook_installed[0] = True


def _build(mask_mode, use_qkb, use_vb):
    """Builds the Bass program. mask_mode: 'causal' | 'none' | 'general'."""
    import functools
    import concourse.bass as bass
    import concourse.mybir as mybir
    import concourse.tile as tile
    from concourse.masks import make_upper_triangular

    dt = mybir.dt
    f32 = dt.float32
    bf16 = dt.bfloat16
    Exp = mybir.ActivationFunctionType.Exp
    Ln = mybir.ActivationFunctionType.Ln
    Identity = mybir.ActivationFunctionType.Identity
    out_dt = bf16 if OUT_BF16 else f32

    causal = mask_mode == "causal"

    nc = bass.Bass()
    xT_d = nc.dram_tensor("xt", [DIM, T], bf16, kind="ExternalInput")
    wqT_d = nc.dram_tensor("wqt", [DIM, HSL], bf16, kind="ExternalInput")
    wkT_d = nc.dram_tensor("wkt", [DIM, HSL], bf16, kind="ExternalInput")
    wvT_d = nc.dram_tensor("wvt", [DIM, HSL], bf16, kind="ExternalInput")
    woT_d = nc.dram_tensor("wot", [HSL, DIM], bf16, kind="ExternalInput")
    outT_d = nc.dram_tensor("outT", [DIM, T], out_dt, kind="ExternalOutput")
    qb_d = kb_d = vb_d = maskT_d = None
    if use_qkb:
        qb_d = nc.dram_tensor("qb", [HSL], f32, kind="ExternalInput")
        kb_d = nc.dram_tensor("kb", [HSL], f32, kind="ExternalInput")
    if use_vb:
        vb_d = nc.dram_tensor("vb", [HSL], f32, kind="ExternalInput")
    if mask_mode == "general":
        maskT_d = nc.dram_tensor("maskt", [SEQ, SEQ], f32, kind="ExternalInput")

    # 3-D views with 128-partition-major layout
    xT_ap = xT_d[:].rearrange("(kt p) t -> p kt t", p=128)
    wq_ap = wqT_d[:].rearrange("(kt p) m -> p kt m", p=128)
    wk_ap = wkT_d[:].rearrange("(kt p) m -> p kt m", p=128)
    wv_ap = wvT_d[:].rearrange("(kt p) m -> p kt m", p=128)
    wo_ap = woT_d[:].rearrange("(g p) n -> p g n", p=128)
    outT_ap = outT_d[:].rearrange("(ot p) t -> p ot t", p=128)

    with tile.TileContext(nc) as tc:
        with (
            tc.tile_pool(name="singles", bufs=1) as singles,
            tc.tile_pool(name="xload", bufs=3) as xload,
            tc.tile_pool(name="work", bufs=4) as work,
            tc.tile_pool(name="outp", bufs=4) as outp,
            tc.tile_pool(name="attps", bufs=2, space="PSUM") as attps,
            tc.tile_pool(name="otps", bufs=2, space="PSUM") as otps,
            tc.tile_pool(name="filps", bufs=2, space="PSUM") as filps,
        ):
            # ---- resident tensors -------------------------------------
            wq_sb = singles.tile([128, NKT, HSL], bf16)
            wk_sb = singles.tile([128, NKT, HSL], bf16)
            wv_sb = singles.tile([128, NKT, HSL], bf16)
            wo_sb = singles.tile([128, 2, DIM], bf16)

            qt_sb = singles.tile([128, 2, T], bf16)
            kt_sb = singles.tile([128, 2, T], bf16)
            ctxT_sb = singles.tile([128, 2, T], bf16)
            # V with 64 ones-columns per head: the P@V matmul then writes the
            # softmax denominator to PSUM partitions 64..127 (a free
            # cross-partition broadcast).
            vaug_sb = singles.tile([128, T // 128, HPC, 2 * HD], bf16)

            # Startup-critical DMAs.  The first compute unit (Q-proj, group
            # 0) needs (wq quarter i, x0 quarter i) pairs in order, so the
            # descriptor generation for the first two pairs is spread over
            # four otherwise-idle engine queues; the rest go on SP.
            xt0 = xload.tile([128, NKT, 512], bf16, tag="xt")
            for q in range(4):
                ksl = slice(4 * q, 4 * q + 4)
                nc.sync.dma_start(out=wq_sb[:, ksl], in_=wq_ap[:, ksl])
                nc.sync.dma_start(out=xt0[:, ksl], in_=xT_ap[:, ksl, 0:512])
            for q in range(4):
                ksl = slice(4 * q, 4 * q + 4)
                nc.sync.dma_start(out=wk_sb[:, ksl], in_=wk_ap[:, ksl])
            for q in range(4):
                ksl = slice(4 * q, 4 * q + 4)
                nc.sync.dma_start(out=wv_sb[:, ksl], in_=wv_ap[:, ksl])
            nc.sync.dma_start(out=wo_sb, in_=wo_ap)



            qb_sb = kb_sb = vb_bc = None
            if use_qkb:
                qb_sb = singles.tile([128, 2], f32)
                kb_sb = singles.tile([128, 2], f32)
                nc.sync.dma_start(out=qb_sb, in_=qb_d[:].rearrange("(g p) -> p g", p=128))
                nc.sync.dma_start(out=kb_sb, in_=kb_d[:].rearrange("(g p) -> p g", p=128))
            if use_vb:
                vb_bc = singles.tile([128, HSL], f32)
                nc.sync.dma_start(out=vb_bc, in_=vb_d[:].to_broadcast([128, HSL]))

            triu_sb = None
            if causal:
                triu_sb = singles.tile([128, 128], bf16)
                make_upper_triangular(nc, triu_sb, val=1.0, diag=True)
                # ~3.4us of dummy matmuls during the initial DMA wait: trips
                # the HAM activity window so the PE is already at 2.4 GHz
                # when the first projection matmuls arrive.
                warm_ps = filps.tile([128, 512], f32, tag="fil", name="warm")
                for w in range(56):
                    nc.tensor.matmul(
                        warm_ps[:, 0:128], lhsT=triu_sb[0:64, :],
                        rhs=triu_sb[0:64, :],
                        start=(w == 0), stop=(w == 55), tile_position=(0, 0))

            # ones columns of V_aug, written once on the DVE (idle at start;
            # keeping it OFF the Pool queue lets the triangular mask finish
            # early so the PE warm-up matmuls fire during the DMA wait)
            nc.vector.memset(vaug_sb[:, :, :, HD:2 * HD], 1.0)

            # ---- filler units -----------------------------------------
            # QKV projections and the output projection are emitted as
            # "filler" thunks interleaved between attention j-steps.  They
            # draw PSUM from their OWN 2-bank pool so they never serialize
            # against the score->exp->PV chain.

            xt_tiles = {0: xt0}

            def load_unit(ch):
                tsl = slice(ch * 512, (ch + 1) * 512)
                xt_ch = xload.tile([128, NKT, 512], bf16, tag="xt")
                for q in range(4):
                    ksl = slice(4 * q, 4 * q + 4)
                    nc.sync.dma_start(out=xt_ch[:, ksl], in_=xT_ap[:, ksl, tsl])
                xt_tiles[ch] = xt_ch

            def qk_unit(ch, w_sb, dst_sb, b_sb, g):
                tsl = slice(ch * 512, (ch + 1) * 512)
                ps = filps.tile([128, 512], f32, tag="fil", name="qk")
                for k in range(NKT):
                    nc.tensor.matmul(
                        ps, lhsT=w_sb[:, k, g * 128:(g + 1) * 128],
                        rhs=xt_tiles[ch][:, k, :],
                        start=(k == 0), stop=(k == NKT - 1))
                if b_sb is not None:
                    nc.scalar.activation(
                        out=dst_sb[:, g, tsl], in_=ps,
                        func=Identity, bias=b_sb[:, g:g + 1], scale=1.0)
                else:
                    nc.vector.tensor_copy(out=dst_sb[:, g, tsl], in_=ps)

            def v_unit(ch, tp):
                ps = filps.tile([128, 512], f32, tag="fil", name="v")
                for i in range(2):
                    tt = 2 * tp + i
                    for k in range(NKT):
                        nc.tensor.matmul(
                            ps[:, i * HSL:(i + 1) * HSL],
                            lhsT=xt_tiles[ch][:, k, tt * 128:(tt + 1) * 128],
                            rhs=wv_sb[:, k, :],
                            start=(k == 0), stop=(k == NKT - 1))
                tg0 = ch * 4 + 2 * tp
                vdst = vaug_sb[:, tg0:tg0 + 2, :, 0:HD]
                vsrc = ps.rearrange("p (i h m) -> p i h m", i=2, h=HPC)
                if vb_bc is not None:
                    nc.vector.tensor_add(
                        out=vdst, in0=vsrc,
                        in1=vb_bc[:, None, :].to_broadcast(
                            [128, 2, HSL]).rearrange(
                            "p i (h m) -> p i h m", h=HPC))
                else:
                    nc.vector.tensor_copy(out=vdst, in_=vsrc)

            def qkv_units(ch, with_load=True):
                """Thunk list for one 512-token chunk of QKV projection."""
                th = []
                if with_load:
                    th.append(functools.partial(load_unit, ch))
                for g in range(2):
                    th.append(functools.partial(qk_unit, ch, wq_sb, qt_sb, qb_sb, g))
                for g in range(2):
                    th.append(functools.partial(qk_unit, ch, wk_sb, kt_sb, kb_sb, g))
                for tp in range(2):
                    th.append(functools.partial(v_unit, ch, tp))
                return th

            osb_box = {}

            def o_unit(ch, o, vec_evict=False, dma_eng=None):
                tsl = slice(ch * 512, (ch + 1) * 512)
                ps = filps.tile([128, 512], f32, tag="fil", name="o")
                for g2 in range(2):
                    nc.tensor.matmul(
                        ps, lhsT=wo_sb[:, g2, o * 128:(o + 1) * 128],
                        rhs=ctxT_sb[:, g2, tsl],
                        start=(g2 == 0), stop=(g2 == 1))
                if o % 2 == 0:
                    osb_box[ch] = outp.tile([128, 2, 512], out_dt,
                                            tag="out_sb", name="osb")
                osb = osb_box[ch]
                if vec_evict or o % 4 < 2:
                    nc.vector.tensor_copy(out=osb[:, o % 2], in_=ps)
                else:
                    nc.scalar.copy(out=osb[:, o % 2], in_=ps)
                if o % 2 == 1:
                    (dma_eng or nc.sync).dma_start(
                        out=outT_ap[:, o - 1:o + 1, tsl], in_=osb)

            def o_pair2(ch, op, vec_evict=False, dma_eng=None):
                """Two output row-blocks on one [128,1024] PSUM tile from the
                attention pool — used only in the final drain, when the
                score pool is free, to widen the eviction rotation."""
                tsl = slice(ch * 512, (ch + 1) * 512)
                ps2 = attps.tile([128, 1024], f32, tag="st2", name="o2")
                for i in range(2):
                    o = 2 * op + i
                    for g2 in range(2):
                        nc.tensor.matmul(
                            ps2[:, i * 512:(i + 1) * 512],
                            lhsT=wo_sb[:, g2, o * 128:(o + 1) * 128],
                            rhs=ctxT_sb[:, g2, tsl],
                            start=(g2 == 0), stop=(g2 == 1))
                osb = outp.tile([128, 2, 512], out_dt, tag="out_sb", name="osb")
                src2 = ps2.rearrange("p (i n) -> p i n", i=2)
                if vec_evict:
                    nc.vector.tensor_copy(out=osb, in_=src2)
                else:
                    nc.scalar.copy(out=osb, in_=src2)
                (dma_eng or nc.sync).dma_start(
                    out=outT_ap[:, 2 * op:2 * op + 2, tsl], in_=osb)

            def oproj_units(ch, vec_evict=False):
                return [functools.partial(o_unit, ch, o, vec_evict)
                        for o in range(DIM // 128)]

            def oproj_drain(ch):
                """Final chunk: the g2=0 half of the first six row-blocks
                only needs head-group 0's context (ready one region early),
                so those matmuls run while the last region's reciprocal
                chain produces head-group 1's context.  PSUM draws from
                both the filler pool and the (now idle) attention pool."""
                tsl = slice(ch * 512, (ch + 1) * 512)
                a = filps.tile([128, 512], f32, tag="fil", name="da")
                b = filps.tile([128, 512], f32, tag="fil", name="db")
                c2 = attps.tile([128, 1024], f32, tag="st2", name="dc")
                d2 = attps.tile([128, 1024], f32, tag="st2", name="dd")
                slots = [(a, [0]), (b, [1]), (c2, [2, 3]), (d2, [4, 5])]
                for g2 in range(2):
                    for ps, olist in slots:
                        for idx, o in enumerate(olist):
                            dst = ps[:, idx * 512:(idx + 1) * 512] \
                                if len(olist) > 1 else ps
                            nc.tensor.matmul(
                                dst,
                                lhsT=wo_sb[:, g2, o * 128:(o + 1) * 128],
                                rhs=ctxT_sb[:, g2, tsl],
                                start=(g2 == 0), stop=(g2 == 1))
                osb_ab = outp.tile([128, 2, 512], out_dt, tag="out_sb",
                                   name="osb")
                nc.vector.tensor_copy(out=osb_ab[:, 0], in_=a)
                nc.scalar.copy(out=osb_ab[:, 1], in_=b)
                nc.sync.dma_start(out=outT_ap[:, 0:2, tsl], in_=osb_ab)
                for si, ps in ((0, c2), (1, d2)):
                    osb = outp.tile([128, 2, 512], out_dt, tag="out_sb",
                                    name="osb")
                    src = ps.rearrange("p (i n) -> p i n", i=2)
                    if si == 0:
                        nc.vector.tensor_copy(out=osb, in_=src)
                    else:
                        nc.scalar.copy(out=osb, in_=src)
                    nc.sync.dma_start(
                        out=outT_ap[:, 2 + 2 * si:4 + 2 * si, tsl], in_=osb)
                # remaining row-blocks, ordinary rotation
                o_unit(ch, 6, True)
                o_unit(ch, 7, False)
                o_pair2(ch, 4, True)
                o_pair2(ch, 5, False)
                o_pair2(ch, 6, True)
                o_pair2(ch, 7, False)

            def pump(filler, n=1):
                for _ in range(n):
                    t = next(filler, None)
                    if t is None:
                        return False
                    t()
                return True

            def drain(filler):
                while pump(filler):
                    pass

            # ---- attention regions ------------------------------------

            pending_ep = [None]

            def att_region(b, c, gg, filler, reserve_n=2, last=False):
                """Attention for one (batch, q-chunk, head-pair): the two
                heads of group gg are row-packed in the score matmuls and
                PSUM tile; O^T flush matmuls lag 3 j-steps; filler thunks
                are spread over the j-steps with 2 reserved to bridge the
                region boundary.  The previous region's 1/den + ctx-scale
                chain is emitted after this region's first exp so it never
                delays the exp stream at the boundary."""
                thunks = list(filler)
                reserve = thunks[-reserve_n:] if len(thunks) > reserve_n else []
                body = thunks[:len(thunks) - len(reserve)]
                bi = [0]
                ots = [otps.tile([128, 512], f32, tag="ot", name="ot")
                       for _ in range(2)]
                jmax = 4 * c + 4 if causal else NJ
                pend = []

                def flush_ot(j, qo, pt2):
                    for hh in range(2):
                        nc.tensor.matmul(
                            ots[hh][:, qo:512],
                            lhsT=vaug_sb[:, b * NJ + j, 2 * gg + hh, :],
                            rhs=pt2[:, 512 * hh + qo:512 * hh + 512],
                            start=(j == 0), stop=(j == jmax - 1))

                for j in range(jmax):
                    qo = max(0, j * 128 - c * 512) if causal else 0
                    ssl = slice(b * SEQ + j * 128, b * SEQ + (j + 1) * 128)
                    qsl = slice(b * SEQ + c * 512 + qo, b * SEQ + (c + 1) * 512)
                    st2 = attps.tile([128, 1024], f32, tag="st2", name="st2")
                    nc.tensor.matmul(
                        st2[:, qo:512], lhsT=kt_sb[0:64, gg, ssl],
                        rhs=qt_sb[0:64, gg, qsl],
                        start=True, stop=True, tile_position=(0, 0))
                    nc.tensor.matmul(
                        st2[:, 512 + qo:1024], lhsT=kt_sb[64:128, gg, ssl],
                        rhs=qt_sb[64:128, gg, qsl],
                        start=True, stop=True, tile_position=(64, 0))
                    if maskT_d is not None:
                        mt = work.tile([128, 512], f32, tag="mt")
                        nc.sync.dma_start(
                            out=mt,
                            in_=maskT_d[j * 128:(j + 1) * 128,
                                        c * 512:(c + 1) * 512])
                        for hh in range(2):
                            sl = slice(512 * hh, 512 * hh + 512)
                            nc.vector.tensor_add(
                                out=st2[:, sl], in0=st2[:, sl], in1=mt)
                    pt2 = work.tile([128, 1024], bf16, tag="pt", bufs=8)
                    nc.scalar.activation(
                        out=pt2.rearrange("p (two n) -> p two n", two=2)[:, :, qo:512],
                        in_=st2.rearrange("p (two n) -> p two n", two=2)[:, :, qo:512],
                        func=Exp, scale=SCALE)
                    if causal and j * 128 >= c * 512:
                        dv = pt2.rearrange("p (two n) -> p two n", two=2)[:, :, qo:qo + 128]
                        nc.vector.tensor_mul(
                            out=dv, in0=dv,
                            in1=triu_sb[:, None, :].to_broadcast([128, 2, 128]))
                    pend.append((j, qo, pt2))
                    if j == 0 and pending_ep[0] is not None:
                        pending_ep[0]()
                        pending_ep[0] = None
                    while len(pend) > (1 if last else 3):
                        flush_ot(*pend.pop(0))
                    # front-load two units so the boundary (where the exp
                    # chain restarts and the deferred recip runs) has PE work
                    want = ((j + 1) * len(body) + jmax - 1) // jmax + 2
                    while bi[0] < min(want, len(body)):
                        body[bi[0]]()
                        bi[0] += 1
                while pend:
                    flush_ot(*pend.pop(0))
                for t in reserve:
                    t()
                # region end: one f32 copy per accumulator frees its PSUM
                # bank immediately; the 1/den = exp(-ln(den)) + ctx scale
                # run later (deferred past the next region's first exp) from
                # SBUF, batched over both heads.
                csl = slice(b * SEQ + c * 512, b * SEQ + (c + 1) * 512)
                un2 = work.tile([128, 2, 512], f32, tag="unctx", bufs=2)
                nc.vector.tensor_copy(out=un2[:, 0], in_=ots[0])
                if last:
                    # parallel eviction across engines shortens the final
                    # serial chain into the output-projection drain
                    nc.scalar.copy(out=un2[:, 1], in_=ots[1])
                else:
                    nc.vector.tensor_copy(out=un2[:, 1], in_=ots[1])

                def epilogue():
                    rb2 = work.tile([64, 2, 512], f32, tag="rb", bufs=2)
                    nc.scalar.activation(out=rb2, in_=un2[HD:2 * HD],
                                         func=Ln, scale=1.0)
                    nc.scalar.activation(out=rb2, in_=rb2, func=Exp, scale=-1.0)
                    for hh in range(2):
                        nc.vector.tensor_mul(
                            out=ctxT_sb[hh * 64:(hh + 1) * 64, gg, csl],
                            in0=un2[0:HD, hh], in1=rb2[:, hh])

                pending_ep[0] = epilogue

            # ---- schedule ---------------------------------------------
            # qkv chunks feed forward (region (b,c) needs chunks <= 4b+c);
            # fine-grained oproj units land in the ScalarE-heavy late
            # regions.  Each entry: (b, c, gg, filler thunks).
            def mix(units, ounits):
                """Interleave o-units between the bigger qkv units so their
                eviction latency hides under the 16-matmul streams."""
                out = []
                per = (len(ounits) + len(units) - 1) // max(len(units), 1)
                oi = 0
                for u in units:
                    out.append(u)
                    for _ in range(per):
                        if oi < len(ounits):
                            out.append(ounits[oi])
                            oi += 1
                out.extend(ounits[oi:])
                return out

            q1, q2, q3 = qkv_units(1), qkv_units(2), qkv_units(3)
            q4, q5 = qkv_units(4), qkv_units(5)
            q6, q7 = qkv_units(6), qkv_units(7)
            o0, o1 = oproj_units(0), oproj_units(1)
            o2 = oproj_units(2, vec_evict=True)
            o3 = oproj_units(3, vec_evict=True)
            o4 = oproj_units(4, vec_evict=True)
            o5 = oproj_units(5, vec_evict=True)
            o6 = oproj_units(6, vec_evict=True)
            # chunk X's ctx epilogue is deferred into the region AFTER X's
            # last one, so oproj(X) units are placed two or more regions
            # after region X to avoid stalling the in-order PE queue.
            regions = [
                (0, 0, 0, q1[:4], 2),
                (0, 0, 1, q1[4:], 2),
                (0, 1, 0, q2[:4], 2),
                (0, 1, 1, mix(q2[4:], o0[:8]), 2),
                (0, 2, 0, mix(q3[:4], o0[8:]), 2),
                (0, 2, 1, mix(q3[4:] + q4[:1], o1[:8]), 2),
                (0, 3, 0, mix(q4[1:4], o1[8:]), 2),
                (0, 3, 1, mix(q4[4:] + q5[:2], o2[:8]), 2),
                (1, 0, 0, mix(q5[2:5], o2[8:]), 2),
                (1, 0, 1, q5[5:] + q6[:2], 2),
                (1, 1, 0, q6[2:], 2),
                (1, 1, 1, q7, 2),
                (1, 2, 0, o3 + o4[:8], 4),
                (1, 2, 1, o4[8:] + o5[:8], 4),
                (1, 3, 0, o5[8:] + o6[:4], 4),
                (1, 3, 1, o6[4:10], 2),
            ]
            drain(iter(qkv_units(0, with_load=False)))
            for ri, (b, c, gg, filler, rn) in enumerate(regions):
                att_region(b, c, gg, filler, reserve_n=rn,
                           last=(ri == len(regions) - 1))
            # the last region's recip chain (emitted first so its DVE muls
            # sit ahead of the bridge evictions in the Vector FIFO) overlaps
            # the leftover chunk-6 units and the g2=0 half of the chunk-7
            # drain, neither of which depends on it.
            if pending_ep[0] is not None:
                pending_ep[0]()
                pending_ep[0] = None
            if causal:
                # Dummy matmuls into an accumulator-pool tile: the pool
                # rotation makes them wait for the epilogue's eviction copy,
                # so they execute exactly inside the recip-chain bubble and
                # keep the PE's HAM activity window warm through it.
                dummy = otps.tile([128, 512], f32, tag="ot", name="hamwarm")
                for w in range(64):
                    nc.tensor.matmul(
                        dummy[:, 0:128], lhsT=triu_sb[0:64, :],
                        rhs=triu_sb[0:64, :],
                        start=(w == 0), stop=(w == 63), tile_position=(0, 0))
            for t in o6[10:]:
                t()
            oproj_drain(7)

    return nc


def _classify_mask(mask):
    m = np.asarray(mask, dtype=np.float32).reshape(SEQ, SEQ)
    if not np.any(m):
        return "none", None
    lower_ok = not np.any(m[np.tril_indices(SEQ)])
    upper = m[np.triu_indices(SEQ, 1)]
    if lower_ok and np.all(np.isneginf(upper)):
        return "causal", None
    return "general", np.ascontiguousarray(m.T)


def kernel(x, start_pos, freqs_cis, mask, wq_w, wq_b, wk_w, wk_b,
           wv_w, wv_b, wo_w, wo_b):
    global LAST_RESULTS
    _install_compile_hook()
    from concourse.bass_utils import run_bass_kernel_spmd

    x = np.asarray(x, dtype=np.float32)
    mask_mode, maskT = _classify_mask(mask)
    wq_b = np.asarray(wq_b, dtype=np.float32)
    wk_b = np.asarray(wk_b, dtype=np.float32)
    wv_b = np.asarray(wv_b, dtype=np.float32)
    wo_b = np.asarray(wo_b, dtype=np.float32)
    use_qkb = bool(np.any(wq_b) or np.any(wk_b))
    use_vb = bool(np.any(wv_b))

    nc = _build(mask_mode, use_qkb, use_vb)

    xT = np.ascontiguousarray(x.reshape(T, DIM).T).astype(BF16)
    wqT = np.asarray(wq_w, dtype=np.float32).T.astype(BF16)  # [D, D]
    wkT = np.asarray(wk_w, dtype=np.float32).T.astype(BF16)
    wvT = np.asarray(wv_w, dtype=np.float32).T.astype(BF16)
    wo = np.asarray(wo_w, dtype=np.float32)

    in_maps = []
    for c in range(NCORES):
        sl = slice(HSL * c, HSL * (c + 1))
        im = {
            "xt": xT,
            "wqt": np.ascontiguousarray(wqT[:, sl]),
            "wkt": np.ascontiguousarray(wkT[:, sl]),
            "wvt": np.ascontiguousarray(wvT[:, sl]),
            "wot": np.ascontiguousarray(wo[:, sl].T).astype(BF16),
        }
        if use_qkb:
            im["qb"] = np.ascontiguousarray(wq_b[sl])
            im["kb"] = np.ascontiguousarray(wk_b[sl])
        if use_vb:
            im["vb"] = np.ascontiguousarray(wv_b[sl])
        if mask_mode == "general":
            im["maskt"] = maskT
        in_maps.append(im)

    res = run_bass_kernel_spmd(nc, in_maps, core_ids=list(range(NCORES)))
    LAST_RESULTS = res

    acc = np.zeros((DIM, T), dtype=np.float32)
    for r in res.results:
        acc += np.asarray(r["outT"], dtype=np.float32)
    out = acc.T + wo_b[None, :]
    return out.reshape(BSZ, SEQ, DIM).astype(np.float32)
